# revision 1
# baseline (speedup 1.0000x reference)
"""Galerkin attention (ragged graph segments) on 8 Trainium2 NeuronCores.

Math (per reference):
  qkv = x @ w_qkv.T ; split q,k,v -> [B, H, N, DH]
  k, v  <- LayerNorm over DH (eps=1e-6, affine)
  per graph g (sorted contiguous segments of N): ktv[g] = k_g^T v_g
  out_n = (q_n / size(g(n))) @ ktv[g(n)]
  y = out @ w_out.T + b_out

Sharding: 32 graphs are bin-packed onto 8 cores x S slots; every core runs
the identical instruction stream (SPMD) over T = sum(L_s) 128-row tiles per
batch entry, where L_s is the max tile count of slot s across cores. Ragged
graph ends are zero-padded; padding is neutralized by folding a 0/1 mask
into the LayerNorm affine scalars (k,v side) and folding mask/size(g) into
the final per-node output scale (q side).

All large matmuls run as float32r (fp22 multiply, fp32 accumulate, full PE
speed at free-dim>=256); the small ktv accumulations run true fp32.
"""

import os
import sys

if "/opt/trn_rl_repo" not in sys.path:
    sys.path.insert(0, "/opt/trn_rl_repo")

import numpy as np

import concourse.bacc as bacc
import concourse.bass as bass
import concourse.mybir as mybir
import concourse.tile as tile
from concourse.bass_utils import run_bass_kernel_spmd

P = 128
B = 2
DIM = 512
HEADS = 8
DH = 64
INNER = HEADS * DH          # 512
R = 3 * INNER               # 1536
NCH = DIM // P              # 4 contraction chunks
NPAIRS = HEADS // 2         # 4 head pairs
EPS = 1e-6
N_CORES = 8
GRP = 4                     # tiles per matmul group (512-node span)
F32 = mybir.dt.float32
F32R = mybir.dt.float32r

_PROGRAM_CACHE: dict = {}


def _r(ap):
    return ap.bitcast(F32R)


# ---------------------------------------------------------------------------
# host-side planning
# ---------------------------------------------------------------------------

def _plan(batch, num_graphs, n_cores):
    """Assign graphs to (core, slot) and compute the uniform slot widths."""
    batch = np.asarray(batch).astype(np.int64)
    G = int(num_graphs)
    counts = np.bincount(batch, minlength=G)[:G].astype(np.int64)
    starts = np.concatenate([[0], np.cumsum(counts)[:-1]])
    tiles_g = (counts + P - 1) // P

    S = (G + n_cores - 1) // n_cores
    order = np.argsort(-tiles_g, kind="stable")
    core_graphs = [[] for _ in range(n_cores)]
    core_load = [0] * n_cores
    for g in order:
        cands = [c for c in range(n_cores) if len(core_graphs[c]) < S]
        c = min(cands, key=lambda cc: (core_load[cc], cc))
        core_graphs[c].append(int(g))
        core_load[c] += int(tiles_g[g])
    for c in range(n_cores):
        core_graphs[c].sort(key=lambda g: -int(tiles_g[g]))
        while len(core_graphs[c]) < S:
            core_graphs[c].append(-1)

    Ls = []
    for s in range(S):
        L = max(
            int(tiles_g[core_graphs[c][s]]) if core_graphs[c][s] >= 0 else 0
            for c in range(n_cores)
        )
        Ls.append(max(L, 1))
    return counts, starts, core_graphs, Ls


def _pack_inputs(x, counts, starts, core_graphs, Ls, n_cores):
    T = sum(Ls)
    slot_off = np.concatenate([[0], np.cumsum(Ls)[:-1]])
    xT = np.ascontiguousarray(np.transpose(x, (0, 2, 1)))  # [B, DIM, N]
    per_core = []
    for c in range(n_cores):
        xTp = np.zeros((B, DIM, T * P), np.float32)
        qsc = np.zeros((T * P,), np.float32)
        kvm = np.zeros((T * P,), np.float32)
        for s, g in enumerate(core_graphs[c]):
            if g < 0 or counts[g] == 0:
                continue
            n0, ng = int(starts[g]), int(counts[g])
            off = int(slot_off[s]) * P
            xTp[:, :, off:off + ng] = xT[:, :, n0:n0 + ng]
            qsc[off:off + ng] = 1.0 / ng
            kvm[off:off + ng] = 1.0
        per_core.append((xTp, qsc, kvm))
    return per_core, slot_off


# ---------------------------------------------------------------------------
# device program
# ---------------------------------------------------------------------------

def _build_program(T, Ls, n_cores, ln_general, bo_zero=False):
    from contextlib import ExitStack

    nc = bacc.Bacc("TRN2", target_bir_lowering=False, debug=False,
                   num_devices=n_cores)

    xT = nc.dram_tensor("xT", [B, DIM, T * P], F32R, kind="ExternalInput")
    wq = nc.dram_tensor("wqkvT", [DIM, R], F32R, kind="ExternalInput")
    wo = nc.dram_tensor("woutT", [INNER, DIM], F32R, kind="ExternalInput")
    bo = nc.dram_tensor("bout", [DIM], F32, kind="ExternalInput")
    qsc = nc.dram_tensor("qsc", [T * P], F32, kind="ExternalInput")
    kvm = nc.dram_tensor("kvm", [T * P], F32, kind="ExternalInput")
    if ln_general:
        lnp = nc.dram_tensor("lnp", [4, DH], F32, kind="ExternalInput")
    out = nc.dram_tensor("out", [B, T * P, DIM], F32, kind="ExternalOutput")

    Lmax = max(Ls)
    slot_off = [0]
    for L in Ls[:-1]:
        slot_off.append(slot_off[-1] + L)

    Sqrt = mybir.ActivationFunctionType.Sqrt
    mult = mybir.AluOpType.mult
    add = mybir.AluOpType.add

    with ExitStack() as ctx:
        tc = ctx.enter_context(tile.TileContext(nc))
        const = ctx.enter_context(tc.tile_pool(name="const", bufs=1))

        WQ = const.tile([P, NCH, R], F32R, tag="WQ")
        nc.sync.dma_start(out=WQ[:], in_=wq.ap().rearrange("(k c) r -> c k r", c=P))
        WO = const.tile([P, NCH, DIM], F32R, tag="WO")
        nc.sync.dma_start(out=WO[:], in_=wo.ap().rearrange("(k c) d -> c k d", c=P))
        BO = const.tile([P, DIM], F32, tag="BO")
        nc.sync.dma_start(out=BO[:], in_=bo.ap().partition_broadcast(P))
        QS = const.tile([P, T], F32, tag="QS")
        nc.sync.dma_start(out=QS[:], in_=qsc.ap().rearrange("(t p) -> p t", p=P))
        KM = const.tile([P, T], F32, tag="KM")
        nc.sync.dma_start(out=KM[:], in_=kvm.ap().rearrange("(t p) -> p t", p=P))
        EPSC = const.tile([P, 1], F32, tag="EPSC")
        nc.vector.memset(EPSC[:], EPS)
        if ln_general:
            LNP = const.tile([P, 4, DH], F32, tag="LNP")
            nc.sync.dma_start(out=LNP[:], in_=lnp.ap().partition_broadcast(P))

        xpool = ctx.enter_context(tc.tile_pool(name="xp", bufs=3))
        kvsb = ctx.enter_context(tc.tile_pool(name="kvsb", bufs=3))
        stat = ctx.enter_context(tc.tile_pool(name="stat", bufs=4))
        qstash = ctx.enter_context(tc.tile_pool(name="qstash", bufs=2 * NPAIRS))
        bdsb = ctx.enter_context(tc.tile_pool(name="bd", bufs=2))
        ohsb = ctx.enter_context(tc.tile_pool(name="oh", bufs=2 * NPAIRS))
        outsb = ctx.enter_context(tc.tile_pool(name="outsb", bufs=3))

        kvps = ctx.enter_context(tc.tile_pool(name="kvps", bufs=2, space="PSUM"))
        qtps = ctx.enter_context(tc.tile_pool(name="qtps", bufs=1, space="PSUM"))
        ktps = ctx.enter_context(tc.tile_pool(name="ktps", bufs=1, space="PSUM"))
        mips = ctx.enter_context(tc.tile_pool(name="mips", bufs=1, space="PSUM"))

        for b in range(B):
            for s, L in enumerate(Ls):
                soff = slot_off[s]
                ktv = ktps.tile([P, 2, NPAIRS, P], F32, tag="ktv")
                qts = [qstash.tile([P, L * P], F32R, name=f"qts{i}", tag="qstash")
                       for i in range(NPAIRS)]
                ngroups = (L + GRP - 1) // GRP

                # ---- phase 1: qkv projection, LN(k,v), ktv accumulation ----
                for grp in range(ngroups):
                    gt0 = grp * GRP
                    gw = min(GRP, L - gt0)
                    GW = gw * P
                    n0 = (soff + gt0) * P

                    xt = xpool.tile([P, NCH, GW], F32R, tag="xt")
                    nc.sync.dma_start(
                        out=xt[:],
                        in_=xT.ap()[b].rearrange("(k c) n -> c k n", c=P)[:, :, n0:n0 + GW],
                    )

                    # q^T directly: stationary = W_q pair block, moving = x^T
                    for p in range(NPAIRS):
                        qtp = qtps.tile([P, GW], F32, tag="qtp")
                        for k in range(NCH):
                            nc.tensor.matmul(
                                qtp[:],
                                lhsT=WQ[:, k, p * P:(p + 1) * P],
                                rhs=xt[:, k, :],
                                start=(k == 0), stop=(k == NCH - 1),
                            )
                        nc.scalar.copy(out=qts[p][:, gt0 * P:gt0 * P + GW], in_=qtp[:])

                    for tl in range(gw):
                        t = gt0 + tl
                        ti = soff + t  # global tile index (mask/scale column)

                        kv = kvps.tile([P, 2 * INNER], F32, tag="kv")
                        for k in range(NCH):
                            lx = xt[:, k, tl * P:(tl + 1) * P]
                            nc.tensor.matmul(
                                kv[:, 0:INNER], lhsT=lx,
                                rhs=WQ[:, k, INNER:2 * INNER],
                                start=(k == 0), stop=(k == NCH - 1))
                            nc.tensor.matmul(
                                kv[:, INNER:2 * INNER], lhsT=lx,
                                rhs=WQ[:, k, 2 * INNER:R],
                                start=(k == 0), stop=(k == NCH - 1))

                        # LN statistics for k and v (per node, per head).
                        # bn_stats must see exactly one group per call (AP
                        # optimization merges contiguous groups otherwise).
                        st = stat.tile([P, 2, HEADS, 6], F32, tag="st")
                        kv3 = kv[:].rearrange("p (t h d) -> p t h d", t=2, h=HEADS)
                        for h in range(HEADS):
                            nc.vector.bn_stats(out=st[:, 0, h], in_=kv3[:, 0, h])
                            nc.vector.bn_stats(out=st[:, 1, h], in_=kv3[:, 1, h])
                        ag = stat.tile([P, 2, HEADS, 2], F32, tag="ag")
                        for h in range(HEADS):
                            nc.vector.bn_aggr(out=ag[:, 0, h], in_=st[:, 0, h])
                            nc.vector.bn_aggr(out=ag[:, 1, h], in_=st[:, 1, h])

                        std = stat.tile([P, 2, HEADS], F32, tag="std")
                        nc.scalar.activation(std[:], ag[:, :, :, 1], Sqrt,
                                             bias=EPSC[:, 0:1])
                        rstd = stat.tile([P, 2, HEADS], F32, tag="rstd")
                        nc.vector.reciprocal(rstd[:], std[:])
                        # a = rstd * mask ; c = -mu * a   (pad rows -> a=c=0)
                        av = stat.tile([P, 2, HEADS], F32, tag="av")
                        nc.vector.tensor_scalar(av[:], rstd[:], KM[:, ti:ti + 1],
                                                None, op0=mult)
                        cv = stat.tile([P, 2, HEADS], F32, tag="cv")
                        nc.vector.scalar_tensor_tensor(
                            cv[:], ag[:, :, :, 0], -1.0, av[:],
                            op0=mult, op1=mult)

                        kl = kvsb.tile([P, INNER], F32, tag="kl")
                        vl = kvsb.tile([P, INNER], F32, tag="vl")
                        for h in range(HEADS):
                            sl = slice(h * DH, (h + 1) * DH)
                            nc.vector.tensor_scalar(
                                kl[:, sl], kv[:, sl],
                                av[:, 0, h:h + 1], cv[:, 0, h:h + 1],
                                op0=mult, op1=add)
                            nc.vector.tensor_scalar(
                                vl[:, sl], kv[:, INNER + h * DH:INNER + (h + 1) * DH],
                                av[:, 1, h:h + 1], cv[:, 1, h:h + 1],
                                op0=mult, op1=add)
                        if ln_general:
                            # mask the bias so pad rows stay exactly zero
                            bmk = stat.tile([P, 2, DH], F32, tag="bmk")
                            nc.vector.tensor_scalar(bmk[:, 0], LNP[:, 1],
                                                    KM[:, ti:ti + 1], None,
                                                    op0=mult)
                            nc.vector.tensor_scalar(bmk[:, 1], LNP[:, 3],
                                                    KM[:, ti:ti + 1], None,
                                                    op0=mult)
                            for h in range(HEADS):
                                sl = slice(h * DH, (h + 1) * DH)
                                nc.vector.tensor_tensor(kl[:, sl], kl[:, sl],
                                                        LNP[:, 0], op=mult)
                                nc.vector.tensor_tensor(kl[:, sl], kl[:, sl],
                                                        bmk[:, 0], op=add)
                                nc.vector.tensor_tensor(vl[:, sl], vl[:, sl],
                                                        LNP[:, 2], op=mult)
                                nc.vector.tensor_tensor(vl[:, sl], vl[:, sl],
                                                        bmk[:, 1], op=add)

                        # ktv[g] += k^T v  (even head -> psum rows 0:64,
                        # odd head -> rows 64:128 via output col-group)
                        # start=True clears has_written for the WHOLE bank,
                        # so exactly one start (first mm of slot) and one
                        # stop (last mm of slot); per-element has_written
                        # bits give the other 7 chains first-write semantics.
                        for p in range(NPAIRS):
                            he, ho = 2 * p, 2 * p + 1
                            nc.tensor.matmul(
                                ktv[0:DH, 0, p, 0:DH],
                                lhsT=kl[:, he * DH:(he + 1) * DH],
                                rhs=vl[:, he * DH:(he + 1) * DH],
                                start=(t == 0 and p == 0),
                                stop=(t == L - 1 and p == NPAIRS - 1))
                            nc.tensor.matmul(
                                ktv[DH:P, 1, p, DH:P],
                                lhsT=kl[:, ho * DH:(ho + 1) * DH],
                                rhs=vl[:, ho * DH:(ho + 1) * DH],
                                start=(t == 0 and p == 0),
                                stop=(t == L - 1 and p == NPAIRS - 1))

                # ---- block-diagonal ktv for the pair-batched phase 2 ----
                bd = bdsb.tile([P, NPAIRS, P], F32R, tag="bd")
                nc.vector.memset(bd[:].bitcast(mybir.dt.uint32), 0)
                for p in range(NPAIRS):
                    nc.vector.tensor_copy(bd[0:DH, p, 0:DH],
                                          ktv[0:DH, 0, p, 0:DH])
                    nc.vector.tensor_copy(bd[DH:P, p, DH:P],
                                          ktv[DH:P, 1, p, DH:P])

                # ---- phase 2: out = (q/size) @ ktv ; y = out @ w_out.T + b ----
                for grp in range(ngroups):
                    gt0 = grp * GRP
                    gw = min(GRP, L - gt0)
                    GW = gw * P

                    ohs = []
                    for p in range(NPAIRS):
                        oh = mips.tile([P, GW], F32, tag="mi")
                        nc.tensor.matmul(
                            oh[:], lhsT=bd[:, p, :],
                            rhs=qts[p][:, gt0 * P:gt0 * P + GW],
                            start=True, stop=True)
                        os_ = ohsb.tile([P, GW], F32R, name="ohs", tag="oh")
                        nc.scalar.copy(out=os_[:], in_=oh[:])
                        ohs.append(os_)

                    for tl in range(gw):
                        t = gt0 + tl
                        ti = soff + t
                        ops = mips.tile([P, DIM], F32, tag="mi")
                        for p in range(NPAIRS):
                            nc.tensor.matmul(
                                ops[:],
                                lhsT=ohs[p][:, tl * P:(tl + 1) * P],
                                rhs=WO[:, p, :],
                                start=(p == 0), stop=(p == NPAIRS - 1))
                        ot = outsb.tile([P, DIM], F32, tag="ot")
                        if bo_zero:
                            # b_out == 0: ACT Copy with per-partition scale
                            # keeps this 512-wide pass off the busy DVE
                            nc.scalar.mul(ot[:], ops[:], QS[:, ti:ti + 1])
                        else:
                            nc.vector.scalar_tensor_tensor(
                                ot[:], ops[:], QS[:, ti:ti + 1], BO[:],
                                op0=mult, op1=add)
                        nc.sync.dma_start(
                            out=out.ap()[b, (soff + t) * P:(soff + t + 1) * P, :],
                            in_=ot[:])

    nc.compile()
    return nc


# ---------------------------------------------------------------------------
# entry point
# ---------------------------------------------------------------------------

def _run(x, w_qkv, ln1_w, ln1_b, ln2_w, ln2_b, w_out, b_out, batch,
         num_graphs, n_cores=N_CORES, trace=False):
    x = np.ascontiguousarray(np.asarray(x, np.float32))
    counts, starts, core_graphs, Ls = _plan(batch, num_graphs, n_cores)
    per_core, slot_off = _pack_inputs(x, counts, starts, core_graphs, Ls, n_cores)
    T = sum(Ls)

    ln1_w = np.asarray(ln1_w, np.float32)
    ln1_b = np.asarray(ln1_b, np.float32)
    ln2_w = np.asarray(ln2_w, np.float32)
    ln2_b = np.asarray(ln2_b, np.float32)
    ln_general = not (
        np.all(ln1_w == 1.0) and np.all(ln1_b == 0.0)
        and np.all(ln2_w == 1.0) and np.all(ln2_b == 0.0)
    )

    bout_np = np.asarray(b_out, np.float32)
    bo_zero = bool(np.all(bout_np == 0.0))
    key = (T, tuple(Ls), n_cores, ln_general, bo_zero)
    nc = _PROGRAM_CACHE.get(key)
    if nc is None:
        nc = _build_program(T, tuple(Ls), n_cores, ln_general, bo_zero)
        _PROGRAM_CACHE[key] = nc

    wqkvT = np.ascontiguousarray(np.asarray(w_qkv, np.float32).T)
    woutT = np.ascontiguousarray(np.asarray(w_out, np.float32).T)
    bout = np.ascontiguousarray(np.asarray(b_out, np.float32))
    lnp = np.stack([ln1_w, ln1_b, ln2_w, ln2_b])

    in_maps = []
    for c in range(n_cores):
        xTp, qscv, kvmv = per_core[c]
        m = {"xT": xTp, "wqkvT": wqkvT, "woutT": woutT, "bout": bout,
             "qsc": qscv, "kvm": kvmv}
        if ln_general:
            m["lnp"] = lnp
        in_maps.append(m)

    res = run_bass_kernel_spmd(nc, in_maps, list(range(n_cores)), trace=trace)

    N = x.shape[1]
    y = np.empty((B, N, DIM), np.float32)
    for c in range(n_cores):
        oc = res.results[c]["out"]
        for s, g in enumerate(core_graphs[c]):
            if g < 0 or counts[g] == 0:
                continue
            n0, ng = int(starts[g]), int(counts[g])
            off = int(slot_off[s]) * P
            y[:, n0:n0 + ng, :] = oc[:, off:off + ng, :]
    return y, res


def kernel(**inputs):
    trace = bool(os.environ.get("GALERKIN_TRACE"))
    y, _ = _run(
        inputs["x"], inputs["w_qkv"], inputs["ln1_w"], inputs["ln1_b"],
        inputs["ln2_w"], inputs["ln2_b"], inputs["w_out"], inputs["b_out"],
        inputs["batch"], inputs["num_graphs"], trace=trace,
    )
    return y



# revision 3
# speedup vs baseline: 1.1076x; 1.1076x over previous
"""Galerkin attention (ragged graph segments) on 8 Trainium2 NeuronCores.

Math (per reference):
  qkv = x @ w_qkv.T ; split q,k,v -> [B, H, N, DH]
  k, v  <- LayerNorm over DH (eps=1e-6, affine)
  per graph g (sorted contiguous segments of N): ktv[g] = k_g^T v_g
  out_n = (q_n / size(g(n))) @ ktv[g(n)]
  y = out @ w_out.T + b_out

Sharding: 32 graphs are bin-packed onto 8 cores x S slots; every core runs
the identical instruction stream (SPMD) over T = sum(L_s) 128-row tiles per
batch entry, where L_s is the max tile count of slot s across cores. Ragged
graph ends are zero-padded; padding is neutralized by folding a 0/1 mask
into the LN scale (a = mask/std) and the final per-node output scale.

v2 dataflow per 128-node tile (engines balanced, PE kept streaming):
  PE : qkv projection (f32r, 512-free), full-cross pair k^T v (fp32,
       128-free, single PSUM bank per slot), per-slot Mf = ktv @ w_out.T,
       phase-2 out = q^T.T @ Mf.
  Act: squares for LN variance, sqrt(var+eps), psum->sbuf copies, out scale.
  DVE: two multi-group tensor_reduce (sum k / sum k^2 per head; reversed
       inner stride defeats the AP contiguity merge), tiny fixups, and the
       broadcast LN multiply (per-node-head scale via stride-0 AP).
  Pool(GpSimd): broadcast LN add (SBUF only; GPSIMD cannot touch PSUM).
Phase 2 of slot s-1 is emitted interleaved into phase 1 of slot s so the
in-order PE queue always has ready matmuls while the LN chain drains.
"""

import os
import sys

if "/opt/trn_rl_repo" not in sys.path:
    sys.path.insert(0, "/opt/trn_rl_repo")

import numpy as np

import concourse.bacc as bacc
import concourse.bass as bass
import concourse.mybir as mybir
import concourse.tile as tile
from concourse.bass_utils import run_bass_kernel_spmd

P = 128
B = 2
DIM = 512
HEADS = 8
DH = 64
INNER = HEADS * DH          # 512
R = 3 * INNER               # 1536
NCH = DIM // P              # 4 contraction chunks
NPAIRS = HEADS // 2         # 4 head pairs
EPS = 1e-6
N_CORES = 8
GRP = 4                     # tiles per matmul group (512-node span)
KTV_LAG = 2                 # tiles of software pipelining before k^T v
F32 = mybir.dt.float32
F32R = mybir.dt.float32r

_PROGRAM_CACHE: dict = {}


def _revap(src):
    """View of `src` with the innermost (contiguous) dim reversed.

    Stats are permutation-invariant; the negative stride stops the AP
    optimizer from merging the per-head groups into one flat run, which
    would turn a multi-group tensor_reduce into a single global one.
    """
    inner = list(src.ap[-1])
    assert inner[0] == 1 and inner[1] == DH
    return bass.AP(
        tensor=src.tensor,
        offset=src.offset + (DH - 1),
        ap=[list(d) for d in src.ap[:-1]] + [[-1, DH]],
    )


# ---------------------------------------------------------------------------
# host-side planning
# ---------------------------------------------------------------------------

def _plan(batch, num_graphs, n_cores):
    """Assign graphs to (core, slot) and compute the uniform slot widths."""
    batch = np.asarray(batch).astype(np.int64)
    G = int(num_graphs)
    counts = np.bincount(batch, minlength=G)[:G].astype(np.int64)
    starts = np.concatenate([[0], np.cumsum(counts)[:-1]])
    tiles_g = (counts + P - 1) // P

    S = (G + n_cores - 1) // n_cores
    order = np.argsort(-tiles_g, kind="stable")
    core_graphs = [[] for _ in range(n_cores)]
    core_load = [0] * n_cores
    for g in order:
        cands = [c for c in range(n_cores) if len(core_graphs[c]) < S]
        c = min(cands, key=lambda cc: (core_load[cc], cc))
        core_graphs[c].append(int(g))
        core_load[c] += int(tiles_g[g])
    for c in range(n_cores):
        core_graphs[c].sort(key=lambda g: -int(tiles_g[g]))
        while len(core_graphs[c]) < S:
            core_graphs[c].append(-1)

    Ls = []
    for s in range(S):
        L = max(
            int(tiles_g[core_graphs[c][s]]) if core_graphs[c][s] >= 0 else 0
            for c in range(n_cores)
        )
        Ls.append(max(L, 1))
    return counts, starts, core_graphs, Ls


def _pack_inputs(x, counts, starts, core_graphs, Ls, n_cores):
    T = sum(Ls)
    slot_off = np.concatenate([[0], np.cumsum(Ls)[:-1]])
    xT = np.ascontiguousarray(np.transpose(x, (0, 2, 1)))  # [B, DIM, N]
    per_core = []
    for c in range(n_cores):
        xTp = np.zeros((B, DIM, T * P), np.float32)
        qsc = np.zeros((T * P,), np.float32)
        kvm = np.zeros((T * P,), np.float32)
        for s, g in enumerate(core_graphs[c]):
            if g < 0 or counts[g] == 0:
                continue
            n0, ng = int(starts[g]), int(counts[g])
            off = int(slot_off[s]) * P
            xTp[:, :, off:off + ng] = xT[:, :, n0:n0 + ng]
            qsc[off:off + ng] = 1.0 / ng
            kvm[off:off + ng] = 1.0
        per_core.append((xTp, qsc, kvm))
    return per_core, slot_off


# ---------------------------------------------------------------------------
# device program
# ---------------------------------------------------------------------------

def _build_program(T, Ls, n_cores, ln_general, bo_zero=False):
    from contextlib import ExitStack

    nc = bacc.Bacc("TRN2", target_bir_lowering=False, debug=False,
                   num_devices=n_cores)

    xT = nc.dram_tensor("xT", [B, DIM, T * P], F32R, kind="ExternalInput")
    wq = nc.dram_tensor("wqkvT", [DIM, R], F32R, kind="ExternalInput")
    wo = nc.dram_tensor("woutT", [INNER, DIM], F32R, kind="ExternalInput")
    bo = nc.dram_tensor("bout", [DIM], F32, kind="ExternalInput")
    qsc = nc.dram_tensor("qsc", [T * P], F32, kind="ExternalInput")
    kvm = nc.dram_tensor("kvm", [T * P], F32, kind="ExternalInput")
    if ln_general:
        lnp = nc.dram_tensor("lnp", [4, DH], F32, kind="ExternalInput")
    out = nc.dram_tensor("out", [B, T * P, DIM], F32, kind="ExternalOutput")

    slot_off = [0]
    for L in Ls[:-1]:
        slot_off.append(slot_off[-1] + L)

    Sqrt = mybir.ActivationFunctionType.Sqrt
    Square = mybir.ActivationFunctionType.Square
    mult = mybir.AluOpType.mult
    add = mybir.AluOpType.add
    sub = mybir.AluOpType.subtract
    X = mybir.AxisListType.X

    with ExitStack() as ctx:
        tc = ctx.enter_context(tile.TileContext(nc))
        const = ctx.enter_context(tc.tile_pool(name="const", bufs=1))

        WQ = const.tile([P, NCH, R], F32R, tag="WQ")
        nc.sync.dma_start(out=WQ[:], in_=wq.ap().rearrange("(k c) r -> c k r", c=P))
        WO = const.tile([P, NCH, DIM], F32R, tag="WO")
        nc.sync.dma_start(out=WO[:], in_=wo.ap().rearrange("(k c) d -> c k d", c=P))
        BO = const.tile([P, DIM], F32, tag="BO")
        nc.sync.dma_start(out=BO[:], in_=bo.ap().partition_broadcast(P))
        QS = const.tile([P, T], F32, tag="QS")
        nc.sync.dma_start(out=QS[:], in_=qsc.ap().rearrange("(t p) -> p t", p=P))
        KM = const.tile([P, T], F32, tag="KM")
        nc.sync.dma_start(out=KM[:], in_=kvm.ap().rearrange("(t p) -> p t", p=P))
        EPSC = const.tile([P, 1], F32, tag="EPSC")
        nc.vector.memset(EPSC[:], EPS)
        if ln_general:
            LNP = const.tile([P, 4, DH], F32, tag="LNP")
            nc.sync.dma_start(out=LNP[:], in_=lnp.ap().partition_broadcast(P))

        xpool = ctx.enter_context(tc.tile_pool(name="xp", bufs=3))
        sqpool = ctx.enter_context(tc.tile_pool(name="sqp", bufs=2))
        klvlp = ctx.enter_context(tc.tile_pool(name="klvlp", bufs=KTV_LAG + 2))
        stat = ctx.enter_context(tc.tile_pool(name="stat", bufs=27))
        qstash = ctx.enter_context(tc.tile_pool(name="qstash", bufs=2 * NPAIRS))
        bdsb = ctx.enter_context(tc.tile_pool(name="bd", bufs=2))
        mfsb = ctx.enter_context(tc.tile_pool(name="mf", bufs=2))
        outsb = ctx.enter_context(tc.tile_pool(name="outsb", bufs=3))

        kvps = ctx.enter_context(tc.tile_pool(name="kvps", bufs=2, space="PSUM"))
        qtps = ctx.enter_context(tc.tile_pool(name="qtps", bufs=1, space="PSUM"))
        ktps = ctx.enter_context(tc.tile_pool(name="ktps", bufs=1, space="PSUM"))
        mips = ctx.enter_context(tc.tile_pool(name="mips", bufs=2, space="PSUM"))

        # phase-2 emitters for the previous slot, interleaved into the next
        # slot's phase 1 to keep the PE queue stocked with ready matmuls
        pending_ph2 = []

        def emit_ph2(k=1):
            for _ in range(k):
                if pending_ph2:
                    pending_ph2.pop(0)()

        def make_ph2(b, soff, t, qts, Mf):
            ti = soff + t

            def go():
                ops = mips.tile([P, DIM], F32, tag="mi")
                for p in range(NPAIRS):
                    nc.tensor.matmul(
                        ops[:],
                        lhsT=qts[p][:, t * P:(t + 1) * P],
                        rhs=Mf[:, p, :],
                        start=(p == 0), stop=(p == NPAIRS - 1),
                    )
                ot = outsb.tile([P, DIM], F32, tag="ot")
                if bo_zero:
                    nc.scalar.mul(ot[:], ops[:], QS[:, ti:ti + 1])
                else:
                    nc.vector.scalar_tensor_tensor(
                        ot[:], ops[:], QS[:, ti:ti + 1], BO[:],
                        op0=mult, op1=add)
                nc.sync.dma_start(
                    out=out.ap()[b, ti * P:(ti + 1) * P, :], in_=ot[:])

            return go

        for b in range(B):
            for s, L in enumerate(Ls):
                soff = slot_off[s]
                ktv = ktps.tile([P, NPAIRS, P], F32, tag="ktv")
                qts = [qstash.tile([P, L * P], F32R, name=f"qts{i}", tag="qstash")
                       for i in range(NPAIRS)]
                ngroups = (L + GRP - 1) // GRP

                # deferred k^T v emitters (pipelined KTV_LAG tiles behind)
                pending_ktv = []

                def emit_ktv():
                    if pending_ktv:
                        pending_ktv.pop(0)()

                for grp in range(ngroups):
                    gt0 = grp * GRP
                    gw = min(GRP, L - gt0)
                    GW = gw * P
                    n0 = (soff + gt0) * P

                    xt = xpool.tile([P, NCH, GW], F32R, tag="xt")
                    nc.sync.dma_start(
                        out=xt[:],
                        in_=xT.ap()[b].rearrange("(k c) n -> c k n", c=P)[:, :, n0:n0 + GW],
                    )

                    # q^T: stationary = W_q pair block, moving = x^T
                    for p in range(NPAIRS):
                        qtp = qtps.tile([P, GW], F32, tag="qtp")
                        for k in range(NCH):
                            nc.tensor.matmul(
                                qtp[:],
                                lhsT=WQ[:, k, p * P:(p + 1) * P],
                                rhs=xt[:, k, :],
                                start=(k == 0), stop=(k == NCH - 1),
                            )
                        nc.scalar.copy(out=qts[p][:, gt0 * P:gt0 * P + GW],
                                       in_=qtp[:])
                        emit_ph2()

                    for tl in range(gw):
                        t = gt0 + tl
                        ti = soff + t  # global tile index (mask/scale column)

                        kv = kvps.tile([P, 2, INNER], F32, tag="kv")
                        for k in range(NCH):
                            lx = xt[:, k, tl * P:(tl + 1) * P]
                            nc.tensor.matmul(
                                kv[:, 0, :], lhsT=lx,
                                rhs=WQ[:, k, INNER:2 * INNER],
                                start=(k == 0), stop=(k == NCH - 1))
                            nc.tensor.matmul(
                                kv[:, 1, :], lhsT=lx,
                                rhs=WQ[:, k, 2 * INNER:R],
                                start=(k == 0), stop=(k == NCH - 1))

                        kv4 = kv[:].rearrange("p t (h d) -> p t h d", h=HEADS)

                        # LN statistics: sum and sum-of-squares per head
                        sq = sqpool.tile([P, 2, HEADS, DH + 4], F32, tag="sq")
                        nc.scalar.activation(sq[:, :, :, 0:DH], kv4, Square)
                        smu = stat.tile([P, 2, HEADS], F32, tag="smu")
                        nc.vector.tensor_reduce(out=smu[:], in_=_revap(kv4),
                                                axis=X, op=add)
                        msq = stat.tile([P, 2, HEADS], F32, tag="msq")
                        nc.vector.tensor_reduce(out=msq[:],
                                                in_=sq[:, :, :, 0:DH],
                                                axis=X, op=add)
                        # mu = smu/64 ; var = msq/64 - mu^2
                        mu = stat.tile([P, 2, HEADS], F32, tag="mu")
                        nc.vector.tensor_scalar(mu[:], smu[:], 1.0 / DH, None,
                                                op0=mult)
                        D2 = stat.tile([P, 2, HEADS], F32, tag="D2")
                        nc.vector.tensor_tensor(D2[:], mu[:], mu[:], op=mult)
                        var = stat.tile([P, 2, HEADS], F32, tag="var")
                        nc.vector.scalar_tensor_tensor(
                            var[:], msq[:], 1.0 / DH, D2[:], op0=mult, op1=sub)
                        stdt = stat.tile([P, 2, HEADS], F32, tag="stdt")
                        nc.scalar.activation(stdt[:], var[:], Sqrt,
                                             bias=EPSC[:, 0:1])
                        rstd = stat.tile([P, 2, HEADS], F32, tag="rstd")
                        nc.vector.reciprocal(rstd[:], stdt[:])
                        # a = rstd * mask ; c = -mu * a  (pad rows -> 0)
                        av = stat.tile([P, 2, HEADS], F32, tag="av")
                        nc.vector.tensor_scalar(av[:], rstd[:],
                                                KM[:, ti:ti + 1], None,
                                                op0=mult)
                        cv = stat.tile([P, 2, HEADS], F32, tag="cv")
                        nc.vector.scalar_tensor_tensor(
                            cv[:], mu[:], -1.0, av[:], op0=mult, op1=mult)

                        # apply: mult on DVE (PSUM read), add on GpSimd (SBUF)
                        klvl = klvlp.tile([P, 2, HEADS, DH], F32, tag="klvl")
                        nc.vector.tensor_tensor(
                            klvl[:], kv4,
                            av[:, :, :, None].broadcast_to([P, 2, HEADS, DH]),
                            op=mult)
                        nc.gpsimd.tensor_tensor(
                            klvl[:], klvl[:],
                            cv[:, :, :, None].broadcast_to([P, 2, HEADS, DH]),
                            op=add)
                        if ln_general:
                            bmk = stat.tile([P, 2, DH], F32, tag="bmk")
                            nc.vector.tensor_scalar(bmk[:, 0], LNP[:, 1],
                                                    KM[:, ti:ti + 1], None,
                                                    op0=mult)
                            nc.vector.tensor_scalar(bmk[:, 1], LNP[:, 3],
                                                    KM[:, ti:ti + 1], None,
                                                    op0=mult)
                            for half, wi in ((0, 0), (1, 2)):
                                nc.vector.tensor_tensor(
                                    klvl[:, half], klvl[:, half],
                                    LNP[:, wi, None, :].broadcast_to(
                                        [P, HEADS, DH]), op=mult)
                                nc.vector.tensor_tensor(
                                    klvl[:, half], klvl[:, half],
                                    bmk[:, half, None, :].broadcast_to(
                                        [P, HEADS, DH]), op=add)

                        # full-cross pair k^T v (transposed: lhsT = v side),
                        # deferred KTV_LAG tiles to hide the LN chain latency
                        def make_ktv(klvl=klvl, t=t):
                            def go():
                                klf = klvl[:, 0].rearrange("p h d -> p (h d)")
                                vlf = klvl[:, 1].rearrange("p h d -> p (h d)")
                                for p in range(NPAIRS):
                                    nc.tensor.matmul(
                                        ktv[:, p, :],
                                        lhsT=vlf[:, p * P:(p + 1) * P],
                                        rhs=klf[:, p * P:(p + 1) * P],
                                        start=(t == 0 and p == 0),
                                        stop=(t == L - 1 and p == NPAIRS - 1))
                            return go

                        pending_ktv.append(make_ktv())
                        if len(pending_ktv) > KTV_LAG:
                            emit_ktv()
                        emit_ph2()

                while pending_ktv:
                    emit_ktv()
                emit_ph2(len(pending_ph2))

                # block-diag (ktv_h)^T for Mf and phase 2
                bd = bdsb.tile([P, NPAIRS, P], F32R, tag="bd")
                nc.gpsimd.memset(bd[:].bitcast(mybir.dt.uint32), 0)
                for p in range(NPAIRS):
                    nc.vector.tensor_copy(bd[0:DH, p, 0:DH],
                                          ktv[0:DH, p, 0:DH])
                    nc.vector.tensor_copy(bd[DH:P, p, DH:P],
                                          ktv[DH:P, p, DH:P])

                # Mf = blockdiag(ktv) @ w_out.T   [INNER-pair rows x DIM]
                Mf = mfsb.tile([P, NPAIRS, DIM], F32R, tag="Mf")
                for p in range(NPAIRS):
                    mfp = mips.tile([P, DIM], F32, tag="mi")
                    nc.tensor.matmul(mfp[:], lhsT=bd[:, p, :],
                                     rhs=WO[:, p, :], start=True, stop=True)
                    nc.scalar.copy(out=Mf[:, p, :], in_=mfp[:])

                for t in range(L):
                    pending_ph2.append(make_ph2(b, soff, t, qts, Mf))

        emit_ph2(len(pending_ph2))

    nc.compile()
    return nc


# ---------------------------------------------------------------------------
# entry point
# ---------------------------------------------------------------------------

def _run(x, w_qkv, ln1_w, ln1_b, ln2_w, ln2_b, w_out, b_out, batch,
         num_graphs, n_cores=N_CORES, trace=False):
    x = np.ascontiguousarray(np.asarray(x, np.float32))
    counts, starts, core_graphs, Ls = _plan(batch, num_graphs, n_cores)
    per_core, slot_off = _pack_inputs(x, counts, starts, core_graphs, Ls, n_cores)
    T = sum(Ls)

    ln1_w = np.asarray(ln1_w, np.float32)
    ln1_b = np.asarray(ln1_b, np.float32)
    ln2_w = np.asarray(ln2_w, np.float32)
    ln2_b = np.asarray(ln2_b, np.float32)
    ln_general = not (
        np.all(ln1_w == 1.0) and np.all(ln1_b == 0.0)
        and np.all(ln2_w == 1.0) and np.all(ln2_b == 0.0)
    )

    bout_np = np.asarray(b_out, np.float32)
    bo_zero = bool(np.all(bout_np == 0.0))
    key = (T, tuple(Ls), n_cores, ln_general, bo_zero)
    nc = _PROGRAM_CACHE.get(key)
    if nc is None:
        nc = _build_program(T, tuple(Ls), n_cores, ln_general, bo_zero)
        _PROGRAM_CACHE[key] = nc

    wqkvT = np.ascontiguousarray(np.asarray(w_qkv, np.float32).T)
    woutT = np.ascontiguousarray(np.asarray(w_out, np.float32).T)
    bout = np.ascontiguousarray(np.asarray(b_out, np.float32))
    lnp = np.stack([ln1_w, ln1_b, ln2_w, ln2_b])

    in_maps = []
    for c in range(n_cores):
        xTp, qscv, kvmv = per_core[c]
        m = {"xT": xTp, "wqkvT": wqkvT, "woutT": woutT, "bout": bout,
             "qsc": qscv, "kvm": kvmv}
        if ln_general:
            m["lnp"] = lnp
        in_maps.append(m)

    res = run_bass_kernel_spmd(nc, in_maps, list(range(n_cores)), trace=trace)

    N = x.shape[1]
    y = np.empty((B, N, DIM), np.float32)
    for c in range(n_cores):
        oc = res.results[c]["out"]
        for s, g in enumerate(core_graphs[c]):
            if g < 0 or counts[g] == 0:
                continue
            n0, ng = int(starts[g]), int(counts[g])
            off = int(slot_off[s]) * P
            y[:, n0:n0 + ng, :] = oc[:, off:off + ng, :]
    return y, res


def kernel(**inputs):
    trace = bool(os.environ.get("GALERKIN_TRACE"))
    y, _ = _run(
        inputs["x"], inputs["w_qkv"], inputs["ln1_w"], inputs["ln1_b"],
        inputs["ln2_w"], inputs["ln2_b"], inputs["w_out"], inputs["b_out"],
        inputs["batch"], inputs["num_graphs"], trace=trace,
    )
    return y
